# revision 10
# baseline (speedup 1.0000x reference)
"""Trainium2 Bass kernel for MultiQueryAttention (B=2, S=2048, H=1024, 16 heads, hd=64).

Sharding: tokens are flattened [B*S]=4096 and split 512/core across 8 cores
(cores 0-3 -> batch 0, cores 4-7 -> batch 1). Each core computes the shared
K/V for its whole batch from a host-transposed, per-core ROTATED copy of
hidden (rolled so column 0 is the core's first query token); key chunks are
processed in rotated order, which is fine since attention sums over keys.
No collectives; the host only slices/rolls inputs and concatenates outputs.

Per-core pipeline (all matmuls fp16, fp32 PSUM):
  kv   = Wkv^T x            : k^T lands in kTA rows 0:64 (DVE) and,
                              partition-shifted by SWDGE, in kTB rows 64:128;
                              v^T reaches vones [kt, 64|1] via DMA transpose.
  mask : multiplies vones rows (keys), zeroing masked keys' v AND the ones
         column, so both the ctx numerator and the softmax denominator drop
         masked keys -- no mask row / additive -inf needed.
  qT   = Wq^T x             : head pair p stays in its natural [qA|qB, T]
                              psum layout; one DVE bias-add writes qT[:,p,:].
  s    = kTA/kTB^T qT       : per (pair, key chunk): two matmuls (head A via
                              kTA=[k;0], head B via kTB=[0;k]) sharing the
                              moving operand.
  pT   = exp(s/8)           : one ACT pass per (pair, chunk) over both heads.
  ctx  = [v|1]^T pT         : ones column gives the denominators (row 64);
                              ctx for chunk j is emitted after scores j+1 so
                              the PE never head-of-line blocks on the exp.
  out  = ctx^T Wo (+bo)     : per-chunk pieces interleave into later pairs;
                              final output is DMAed as fp16 (host casts).
"""
import numpy as np
import ml_dtypes

import concourse.bass as bass
import concourse.bacc as bacc
import concourse.tile as tile
from concourse import mybir
from contextlib import ExitStack

F16 = mybir.dt.float16
F32 = mybir.dt.float32
F8 = mybir.dt.float8e4

import os
DEBUG_DUMP = False
# dev-only timing ablations (wrong numerics, same dependency structure):
#   smallexp / smallscores / smallctx shrink that stage's tile to 64 cols
ABLATE = set(x for x in os.environ.get("KABLATE", "").split(",") if x)

# Problem dims (hardcoded per spec)
B, S, H = 2, 2048, 1024
NH, HD = 16, 64
NCORES = 8
CORES_PER_BATCH = NCORES // B
T = S // CORES_PER_BATCH  # local query tokens per core = 512


def build_nc(reps=1, loop_reps=1):
    P = 128
    nc = bacc.Bacc("TRN2", target_bir_lowering=False, debug=False,
                   num_devices=NCORES)

    xT = nc.dram_tensor("xT", [H, S], F16, kind="ExternalInput").ap()
    wkv = nc.dram_tensor("wkv", [H, P], F16, kind="ExternalInput").ap()
    wq = nc.dram_tensor("wq", [H, H], F16, kind="ExternalInput").ap()
    wo = nc.dram_tensor("wo", [H, H], F16, kind="ExternalInput").ap()
    bq_p = nc.dram_tensor("bq_p", [P, H // P], F32, kind="ExternalInput").ap()
    bkv_p = nc.dram_tensor("bkv_p", [P, 1], F32, kind="ExternalInput").ap()
    bo_r = nc.dram_tensor("bo_r", [1, H], F32, kind="ExternalInput").ap()
    maskp = nc.dram_tensor("maskp", [P, S // P], F16,
                           kind="ExternalInput").ap()
    out = nc.dram_tensor("out", [T, H], F16, kind="ExternalOutput").ap()

    with tile.TileContext(nc) as tc, ExitStack() as ctx:
        sb1 = ctx.enter_context(tc.tile_pool(name="persist", bufs=1))
        sb2 = ctx.enter_context(tc.tile_pool(name="work", bufs=2))
        sb3 = ctx.enter_context(tc.tile_pool(name="ptiles", bufs=6))
        s_psum = ctx.enter_context(
            tc.tile_pool(name="s_psum", bufs=2, space="PSUM"))
        c_psum = ctx.enter_context(
            tc.tile_pool(name="c_psum", bufs=3, space="PSUM"))
        proj_psum = ctx.enter_context(
            tc.tile_pool(name="proj_psum", bufs=1, space="PSUM"))

        def emit():
            for _ in range(reps):
                body(nc, tc, sb1, sb2, sb3, s_psum, c_psum, proj_psum,
                     xT, wkv, wq, wo, bq_p, bkv_p, bo_r, maskp, out)

        if loop_reps > 1:
            with tc.For_i(0, loop_reps, 1):
                emit()
        else:
            emit()

    nc.compile()
    return nc


def body(nc, tc, sb1, sb2, sb3, s_psum, c_psum, proj_psum,
         xT, wkv, wq, wo, bq_p, bkv_p, bo_r, maskp, out):
    P = 128
    FC = H // P          # 8 contraction chunks over hidden
    KC = S // P          # 16 key chunks
    OC = H // P          # 8 out-proj contraction chunks (= head pairs)
    NP = NH // 2         # 8 head pairs
    NT = T // P          # 4 query chunks
    OW = 512
    NO = H // OW         # 2 out-proj column groups
    TW = 512             # x window width (tokens per kv step)
    KPT = TW // P        # key chunks per window = 4
    NW = S // TW         # 4 windows
    scale = 1.0 / float(np.sqrt(HD))

    # ---- DMA: x windows ride the SP queue (window 0 = this core's queries);
    # small early weights ride the ACT queue (done before the first exp);
    # big late weights (wq tail, wo, bob) are issued from the DVE/Pool
    # queues mid-stream so no transfer ever blocks exp/scores issue ----
    xT_r = xT.rearrange("(fo p) t -> p fo t", p=P)
    xTb_sb = sb1.tile([P, FC, S], F16, tag="xTb")

    def x_window(w):
        nc.sync.dma_start(xTb_sb[:, :, TW * w:TW * (w + 1)],
                          xT_r[:, :, TW * w:TW * (w + 1)])

    # first window in two fc-halves so the kv matmuls start sooner
    nc.sync.dma_start(xTb_sb[:, 0:FC // 2, 0:TW], xT_r[:, 0:FC // 2, 0:TW])
    nc.sync.dma_start(xTb_sb[:, FC // 2:FC, 0:TW],
                      xT_r[:, FC // 2:FC, 0:TW])
    wkv_sb = sb1.tile([P, FC, P], F16, tag="wkv")
    nc.gpsimd.dma_start(wkv_sb[:], wkv.rearrange("(fo p) o -> p fo o", p=P))
    wq_r = wq.rearrange("(fo p) o -> p fo o", p=P)
    wq_sb = sb1.tile([P, FC, H], F16, tag="wq")
    nc.gpsimd.dma_start(wq_sb[:, :, 0:P], wq_r[:, :, 0:P])
    bkv_sb = sb1.tile([P, 1], F32, tag="bkv")
    nc.gpsimd.dma_start(bkv_sb[:], bkv_p[:])
    maskp_sb = sb1.tile([P, KC], F16, tag="maskp")
    nc.gpsimd.dma_start(maskp_sb[:], maskp[:])
    bq_sb = sb1.tile([P, OC], F32, tag="bq")
    nc.gpsimd.dma_start(bq_sb[:], bq_p[:])
    wo_sb = sb1.tile([P, FC, H], F16, tag="wo")
    bob_sb = sb1.tile([P, H], F32, tag="bob")

    # ---- persistent SBUF layouts ----
    kTA = sb1.tile([P, KC, P], F16, tag="kTA")   # [k dims | zeros] x keys
    nc.vector.memset(kTA[HD:P, :, :], 0.0)
    kTB = sb1.tile([P, KC, P], F16, tag="kTB")   # [zeros | k dims] x keys
    nc.vector.memset(kTB[0:HD, :, :], 0.0)
    vones = sb1.tile([P, KC, P], F16, tag="vones")  # keys x [v | 1 | pad]
    nc.vector.memset(vones[:, :, HD:HD + 1], 1.0)
    nc.vector.memset(vones[:, :, HD + 1:HD + 2], 0.0)
    qT = sb1.tile([P, NP, T], F16, tag="qT")     # [qA | qB] per pair
    ctx_all = sb1.tile([P, OC, T], F16, tag="ctx_all")
    out_acc = sb1.tile([P, NT, NO, OW], F32, tag="out_acc")
    out16 = sb1.tile([P, NT, NO, OW], F16, tag="out16")

    # ---- projection pieces ----
    def kv_mms(tau, fc0, fc1, pk=None):
        if pk is None:
            pk = proj_psum.tile([P, TW], F32, tag="proj")
        for fc in range(fc0, fc1):
            nc.tensor.matmul(pk[:], wkv_sb[:, fc, :],
                             xTb_sb[:, fc, TW * tau:TW * (tau + 1)],
                             start=(fc == 0), stop=(fc == FC - 1))
        return pk

    def kv_finish(tau, pk):
        c0, c1 = KPT * tau, KPT * (tau + 1)
        # k rows -> kTA directly (aligned); v rows -> vtmp for the transpose
        nc.vector.tensor_tensor(
            kTA[0:HD, c0:c1, :].rearrange("p a b -> p (a b)"),
            pk[0:HD, :], bkv_sb[0:HD, :].to_broadcast((HD, TW)),
            mybir.AluOpType.add)
        vtmp = sb2.tile([P, TW], F16, tag="vtmp")
        nc.vector.tensor_tensor(vtmp[HD:P, :], pk[HD:P, :],
                                bkv_sb[HD:P, :].to_broadcast((HD, TW)),
                                mybir.AluOpType.add)
        nc.gpsimd.dma_start(
            kTB[HD:P, c0:c1, :].rearrange("p a b -> p (a b)"),
            kTA[0:HD, c0:c1, :].rearrange("p a b -> p (a b)"))
        nc.sync.dma_start_transpose(vones[:, c0:c1, 0:HD], vtmp[HD:P, :])
        for c in range(c0, c1):
            nc.vector.tensor_tensor(
                vones[:, c, 0:66], vones[:, c, 0:66],
                maskp_sb[:, c:c + 1].to_broadcast((P, 66)),
                mybir.AluOpType.mult)

    def kv_proj(tau):
        kv_finish(tau, kv_mms(tau, 0, FC))

    def q_mms(p, fc0, fc1, pq=None):
        if pq is None:
            pq = proj_psum.tile([P, T], F32, tag="proj")
        for fc in range(fc0, fc1):
            nc.tensor.matmul(pq[:], wq_sb[:, fc, P * p:P * (p + 1)],
                             xTb_sb[:, fc, 0:T],
                             start=(fc == 0), stop=(fc == FC - 1))
        return pq

    def q_finish(p, pq):
        nc.vector.tensor_tensor(qT[:, p, :], pq[:],
                                bq_sb[:, p:p + 1].to_broadcast((P, T)),
                                mybir.AluOpType.add)

    def out_piece(cc2, g, ncc=2, epi=False):
        # one piece = ncc cc chunks accumulated in PSUM
        tt, oo = g // NO, g % NO
        if epi:  # reuse the (now idle) scores ring so pieces pipeline
            po2 = s_psum.tile([P, T], F32, tag="s", name="po2")
            po = po2[:, 0:OW]
        else:
            po = proj_psum.tile([P, OW], F32, tag="proj")
        for r in range(ncc):
            nc.tensor.matmul(po[:], ctx_all[:, cc2 + r, P * tt:P * (tt + 1)],
                             wo_sb[:, cc2 + r, OW * oo:OW * (oo + 1)],
                             start=(r == 0), stop=(r == ncc - 1))
        if cc2 == 0:
            nc.vector.tensor_tensor(out_acc[:, tt, oo, :], po[:],
                                    bob_sb[:, OW * oo:OW * (oo + 1)],
                                    mybir.AluOpType.add)
        elif cc2 + ncc < OC:
            nc.vector.tensor_tensor(out_acc[:, tt, oo, :], po[:],
                                    out_acc[:, tt, oo, :],
                                    mybir.AluOpType.add)
        else:
            nc.vector.tensor_tensor(out16[:, tt, oo, :], po[:],
                                    out_acc[:, tt, oo, :],
                                    mybir.AluOpType.add)
            eng = nc.sync if g % 2 == 0 else nc.gpsimd
            eng.dma_start(out[P * tt:P * (tt + 1), OW * oo:OW * (oo + 1)],
                          out16[:, tt, oo, :])

    def normalize(p, cpA, cpB):
        # rows 0:64 are ctx^T, row 64 the softmax denominator
        recA = sb2.tile([1, T], F32, tag="rec")
        nc.vector.reciprocal(recA[:], cpA[HD:HD + 1, :])
        rbA = sb2.tile([HD, T], F32, tag="rec_b")
        nc.gpsimd.partition_broadcast(rbA[:], recA[:])
        nc.vector.tensor_tensor(ctx_all[0:HD, p, :], cpA[0:HD, :], rbA[:],
                                mybir.AluOpType.mult)
        recB = sb2.tile([1, T], F32, tag="rec")
        nc.vector.reciprocal(recB[:], cpB[HD:HD + 1, :])
        rbB = sb2.tile([HD, T], F32, tag="rec_b")
        nc.gpsimd.partition_broadcast(rbB[:], recB[:])
        ctmp = sb2.tile([HD, T], F16, tag="ctmp")
        nc.vector.tensor_tensor(ctmp[:], cpB[0:HD, :], rbB[:],
                                mybir.AluOpType.mult)
        nc.gpsimd.dma_start(ctx_all[HD:P, p, :], ctmp[:])

    # ---- prologue: window 0 -> kv(0) and q(0); remaining windows follow
    # the first v-transpose on the SP queue ----
    kv_proj(0)
    nc.gpsimd.dma_start(wq_sb[:, :, P:4 * P], wq_r[:, :, P:4 * P])
    q_finish(0, q_mms(0, 0, FC))
    for w in range(1, NW):
        x_window(w)
    nc.sync.dma_start(bob_sb[:], bo_r.to_broadcast((P, H)))

    # ---- attention: pairs x key chunks, ctx lags scores by two chunks ----
    for p in range(NP):
        cpA = c_psum.tile([P, T], F32, tag="ctx")
        cpB = c_psum.tile([P, T], F32, tag="ctx")
        lagq = []
        kv_pk = None
        q_pq = None
        SW = 64 if "smallscores" in ABLATE else T
        EW = 64 if "smallexp" in ABLATE else 2 * T
        for j in range(KC):
            # both heads' scores in one 2-bank psum tile -> single exp pass
            sp = s_psum.tile([P, 2 * T], F32, tag="s")
            nc.tensor.matmul(sp[:, 0:SW], kTA[:, j, :], qT[:, p, 0:SW],
                             start=True, stop=True)
            nc.tensor.matmul(sp[:, T:T + SW], kTB[:, j, :], qT[:, p, 0:SW],
                             start=True, stop=True)
            pT = sb3.tile([P, 2 * T], F16, tag="pT")
            nc.scalar.activation(pT[:, 0:EW], sp[:, 0:EW],
                                 mybir.ActivationFunctionType.Exp,
                                 scale=scale)
            pTA = pT[:, 0:T]
            pTB = pT[:, T:2 * T]
            # interleaved projection work (keeps PE busy during the exp);
            # the single proj bank is time-shared: kv/q in the first half of
            # the chunk loop, out-proj pieces in the second half
            if p == 0:
                if j in (0, 1, 2, 3, 6, 7):
                    tau = {0: 1, 1: 1, 2: 2, 3: 2, 6: 3, 7: 3}[j]
                    if j % 2 == 0:
                        kv_pk = kv_mms(tau, 0, FC // 2)
                    else:
                        kv_mms(tau, FC // 2, FC, kv_pk)
                        kv_finish(tau, kv_pk)
                if j == 8:
                    nc.sync.dma_start(
                        wo_sb[:], wo.rearrange("(fo p) o -> p fo o", p=P))
                if j == 12:
                    nc.sync.dma_start(wq_sb[:, :, 4 * P:H],
                                      wq_r[:, :, 4 * P:H])
                if j >= 8:
                    q_pq = q_mms(1, j - 8, j - 7, q_pq if j > 8 else None)
                    if j == 15:
                        q_finish(1, q_pq)
            elif "noproj" not in ABLATE:
                if j < 8 and p < NP - 1:
                    q_pq = q_mms(p + 1, j, j + 1, q_pq if j > 0 else None)
                    if j == 7:
                        q_finish(p + 1, q_pq)
                if j >= 8 and p >= 3 and p % 2 == 1:
                    out_piece(p - 3, j - 8)
            lagq.append((pTA, pTB, j))
            CW = 64 if "smallctx" in ABLATE else T
            del pT, pTA, pTB
            if len(lagq) > 2:
                pA, pB, jp = lagq.pop(0)
                nc.tensor.matmul(cpA[0:HD + 1, 0:CW], vones[:, jp, 0:HD + 1],
                                 pA[:, 0:CW], start=(jp == 0), stop=False)
                nc.tensor.matmul(cpB[0:HD + 1, 0:CW], vones[:, jp, 0:HD + 1],
                                 pB[:, 0:CW], start=(jp == 0), stop=False)
        while lagq:
            pA, pB, jp = lagq.pop(0)
            nc.tensor.matmul(cpA[0:HD + 1, 0:CW], vones[:, jp, 0:HD + 1],
                             pA[:, 0:CW], start=(jp == 0), stop=(jp == KC - 1))
            nc.tensor.matmul(cpB[0:HD + 1, 0:CW], vones[:, jp, 0:HD + 1],
                             pB[:, 0:CW], start=(jp == 0), stop=(jp == KC - 1))
        normalize(p, cpA, cpB)

    # ---- final out-projection chunk pair streams straight to DRAM
    # (pipelined through the now-idle scores banks) ----
    for g in range(NT * NO):
        out_piece(OC - 2, g, epi=True)

    if DEBUG_DUMP:
        for name, t in [("dbg_kTA", kTA), ("dbg_kTB", kTB),
                        ("dbg_vones", vones), ("dbg_qT", qT),
                        ("dbg_ctx", ctx_all)]:
            shp = [int(s) for s in t.shape]
            d = nc.dram_tensor(name, shp, F16, kind="ExternalOutput").ap()
            nc.sync.dma_start(d[:], t[:])
        dacc = nc.dram_tensor("dbg_acc", [P, NT, NO, OW], F32,
                              kind="ExternalOutput").ap()
        nc.sync.dma_start(dacc[:], out_acc[:])


# ---------------- host side ----------------

_RUNNER_CACHE = {}


def _get_runner(reps=1):
    key = reps
    if key not in _RUNNER_CACHE:
        from runner import make_runner  # dev only; grading uses the fallback
        nc = build_nc(reps=reps)
        _RUNNER_CACHE[key] = (nc, make_runner(nc, NCORES))
    return _RUNNER_CACHE[key]


def _prep_in_maps(hidden_state, attention_mask, Wq, bq, Wk, bk, Wv, bv, Wo, bo):
    f16 = np.float16
    hid = np.asarray(hidden_state, np.float32)
    mask = np.asarray(attention_mask, np.float32)
    hT = np.ascontiguousarray(hid.transpose(0, 2, 1)).astype(f16)  # [B, H, S]
    wkv = np.concatenate([np.asarray(Wk, np.float32),
                          np.asarray(Wv, np.float32)], axis=1).astype(f16)
    wq_b = np.asarray(Wq, np.float32).astype(f16)
    wo_b = np.asarray(Wo, np.float32).astype(f16)
    bq_p = np.asarray(bq, np.float32).reshape(H // 128, 128).T.copy()
    bkv_p = np.concatenate([np.asarray(bk, np.float32),
                            np.asarray(bv, np.float32)]).reshape(128, 1)
    bo_r = np.asarray(bo, np.float32).reshape(1, H).copy()
    in_maps = []
    for c in range(NCORES):
        b = c // CORES_PER_BATCH
        s0 = (c % CORES_PER_BATCH) * T
        xrot = np.ascontiguousarray(np.roll(hT[b], -s0, axis=1))
        mrot = np.roll(mask[b], -s0).reshape(S // 128, 128).T.copy()
        in_maps.append({
            "xT": xrot, "wkv": wkv, "wq": wq_b, "wo": wo_b,
            "bq_p": bq_p, "bkv_p": bkv_p, "bo_r": bo_r,
            "maskp": mrot.astype(np.float16),
        })
    return in_maps


def kernel(hidden_state, attention_mask, Wq, bq, Wk, bk, Wv, bv, Wo, bo):
    in_maps = _prep_in_maps(hidden_state, attention_mask,
                            Wq, bq, Wk, bk, Wv, bv, Wo, bo)
    try:
        nc, runner = _get_runner()
        args = runner.put(runner.pack(in_maps))
        outs = runner(args)
        res = runner.unpack(outs)
    except ImportError:
        from concourse.bass_utils import run_bass_kernel_spmd
        nc = build_nc()
        res = run_bass_kernel_spmd(nc, in_maps, list(range(NCORES))).results
    full = np.empty((B, S, H), np.float32)
    for c in range(NCORES):
        b = c // CORES_PER_BATCH
        s0 = (c % CORES_PER_BATCH) * T
        full[b, s0:s0 + T] = res[c]["out"].astype(np.float32)
    return full



# revision 21
# speedup vs baseline: 1.1105x; 1.1105x over previous
"""Trainium2 Bass kernel for MultiQueryAttention (B=2, S=2048, H=1024, 16 heads, hd=64).

Sharding: tokens are flattened [B*S]=4096 and split 512/core across 8 cores
(cores 0-3 -> batch 0, cores 4-7 -> batch 1). Each core computes the shared
K/V for its whole batch from a host-transposed, per-core ROTATED copy of
hidden (rolled so column 0 is the core's first query token); key chunks are
processed in rotated order, which is fine since attention sums over keys.
No collectives; the host only slices/rolls inputs and concatenates outputs.

Per-core pipeline (all matmuls fp16, fp32 PSUM):
  kv   = Wkv^T x            : k^T lands in kTA rows 0:64 (DVE) and,
                              partition-shifted by SWDGE, in kTB rows 64:128;
                              v^T reaches vones [kt, 64|1] via DMA transpose.
  mask : multiplies vones rows (keys), zeroing masked keys' v AND the ones
         column, so both the ctx numerator and the softmax denominator drop
         masked keys -- no mask row / additive -inf needed.
  qT   = Wq^T x             : head pair p stays in its natural [qA|qB, T]
                              psum layout; one DVE bias-add writes qT[:,p,:].
  s    = kTA/kTB^T qT       : per (pair, key chunk): two matmuls (head A via
                              kTA=[k;0], head B via kTB=[0;k]) sharing the
                              moving operand.
  pT   = exp(s/8)           : one ACT pass per (pair, chunk) over both heads.
  ctx  = [v|1]^T pT         : ones column gives the denominators (row 64);
                              ctx for chunk j is emitted after scores j+1 so
                              the PE never head-of-line blocks on the exp.
  out  = ctx^T Wo (+bo)     : per-chunk pieces interleave into later pairs;
                              final output is DMAed as fp16 (host casts).
"""
import numpy as np
import ml_dtypes

import concourse.bass as bass
import concourse.bacc as bacc
import concourse.tile as tile
from concourse import mybir
from contextlib import ExitStack

F16 = mybir.dt.float16
F32 = mybir.dt.float32
F8 = mybir.dt.float8e4

import os
DEBUG_DUMP = False
# dev-only timing ablations (wrong numerics, same dependency structure):
#   smallexp / smallscores / smallctx shrink that stage's tile to 64 cols
ABLATE = set(x for x in os.environ.get("KABLATE", "").split(",") if x)

# Problem dims (hardcoded per spec)
B, S, H = 2, 2048, 1024
NH, HD = 16, 64
NCORES = 8
CORES_PER_BATCH = NCORES // B
T = S // CORES_PER_BATCH  # local query tokens per core = 512


def build_nc(reps=1, loop_reps=1):
    P = 128
    nc = bacc.Bacc("TRN2", target_bir_lowering=False, debug=False,
                   num_devices=NCORES)

    xT = nc.dram_tensor("xT", [H, S], F16, kind="ExternalInput").ap()
    wkv = nc.dram_tensor("wkv", [H, P], F16, kind="ExternalInput").ap()
    wq = nc.dram_tensor("wq", [H, H], F16, kind="ExternalInput").ap()
    wo = nc.dram_tensor("wo", [H, H], F16, kind="ExternalInput").ap()
    bq_p = nc.dram_tensor("bq_p", [P, H // P], F32, kind="ExternalInput").ap()
    bkv_p = nc.dram_tensor("bkv_p", [P, 1], F32, kind="ExternalInput").ap()
    bo_r = nc.dram_tensor("bo_r", [1, H], F32, kind="ExternalInput").ap()
    maskp = nc.dram_tensor("maskp", [P, S // P], F16,
                           kind="ExternalInput").ap()
    selc = nc.dram_tensor("selc", [16, NH // 2, P], F16,
                          kind="ExternalInput").ap()
    out = nc.dram_tensor("out", [T, H], F16, kind="ExternalOutput").ap()

    with tile.TileContext(nc) as tc, ExitStack() as ctx:
        sb1 = ctx.enter_context(tc.tile_pool(name="persist", bufs=1))
        sb2 = ctx.enter_context(tc.tile_pool(name="work", bufs=2))
        sb3 = ctx.enter_context(tc.tile_pool(name="ptiles", bufs=6))
        s_psum = ctx.enter_context(
            tc.tile_pool(name="s_psum", bufs=2, space="PSUM"))
        c_psum = ctx.enter_context(
            tc.tile_pool(name="c_psum", bufs=3, space="PSUM"))
        proj_psum = ctx.enter_context(
            tc.tile_pool(name="proj_psum", bufs=1, space="PSUM"))

        def emit():
            for _ in range(reps):
                body(nc, tc, sb1, sb2, sb3, s_psum, c_psum, proj_psum,
                     xT, wkv, wq, wo, bq_p, bkv_p, bo_r, maskp, selc, out)

        if loop_reps > 1:
            with tc.For_i(0, loop_reps, 1):
                emit()
        else:
            emit()

    nc.compile()
    return nc


def body(nc, tc, sb1, sb2, sb3, s_psum, c_psum, proj_psum,
         xT, wkv, wq, wo, bq_p, bkv_p, bo_r, maskp, selc, out):
    P = 128
    FC = H // P          # 8 contraction chunks over hidden
    KC = S // P          # 16 key chunks
    OC = H // P          # 8 out-proj contraction chunks (= head pairs)
    NP = NH // 2         # 8 head pairs
    NT = T // P          # 4 query chunks
    OW = 512
    NO = H // OW         # 2 out-proj column groups
    TW = 512             # x window width (tokens per kv step)
    KPT = TW // P        # key chunks per window = 4
    NW = S // TW         # 4 windows
    scale = 1.0 / float(np.sqrt(HD))

    # ---- DMA: x windows ride the SP queue (window 0 = this core's queries);
    # small early weights ride the ACT queue (done before the first exp);
    # big late weights (wq tail, wo, bob) are issued from the DVE/Pool
    # queues mid-stream so no transfer ever blocks exp/scores issue ----
    xT_r = xT.rearrange("(fo p) t -> p fo t", p=P)
    xTb_sb = sb1.tile([P, FC, S], F16, tag="xTb")

    def x_window(w):
        nc.sync.dma_start(xTb_sb[:, :, TW * w:TW * (w + 1)],
                          xT_r[:, :, TW * w:TW * (w + 1)])

    # first window in two fc-halves so the kv matmuls start sooner
    nc.sync.dma_start(xTb_sb[:, 0:FC // 2, 0:TW], xT_r[:, 0:FC // 2, 0:TW])
    nc.sync.dma_start(xTb_sb[:, FC // 2:FC, 0:TW],
                      xT_r[:, FC // 2:FC, 0:TW])
    wkv_sb = sb1.tile([P, FC, P], F16, tag="wkv")
    nc.gpsimd.dma_start(wkv_sb[:], wkv.rearrange("(fo p) o -> p fo o", p=P))
    wq_r = wq.rearrange("(fo p) o -> p fo o", p=P)
    wq_sb = sb1.tile([P, FC, H], F16, tag="wq")
    nc.gpsimd.dma_start(wq_sb[:, :, 0:P], wq_r[:, :, 0:P])
    bkv_sb = sb1.tile([P, 1], F32, tag="bkv")
    nc.gpsimd.dma_start(bkv_sb[:], bkv_p[:])
    maskp_sb = sb1.tile([P, KC], F16, tag="maskp")
    nc.gpsimd.dma_start(maskp_sb[:], maskp[:])
    bq_sb = sb1.tile([P, OC], F32, tag="bq")
    nc.gpsimd.dma_start(bq_sb[:], bq_p[:])
    wo_sb = sb1.tile([P, FC, H], F16, tag="wo")
    bob_sb = sb1.tile([P, H], F32, tag="bob")

    # ---- persistent SBUF layouts ----
    kTA = sb1.tile([P, KC, P], F16, tag="kTA")   # [k dims | zeros] x keys
    nc.vector.memset(kTA[HD:P, :, :], 0.0)
    kTB = sb1.tile([P, KC, P], F16, tag="kTB")   # [zeros | k dims] x keys
    nc.vector.memset(kTB[0:HD, :, :], 0.0)
    vones = sb1.tile([P, KC, P], F16, tag="vones")  # keys x [v | 1 | pad]
    nc.vector.memset(vones[:, :, HD:HD + 1], 1.0)
    nc.vector.memset(vones[:, :, HD + 1:HD + 2], 0.0)
    qT = sb1.tile([P, NP, T], F16, tag="qT")     # [qA | qB] per pair
    ctx_all = sb1.tile([P, OC, T], F16, tag="ctx_all")
    out_acc = sb1.tile([P, NT, NO, OW], F32, tag="out_acc")
    out16 = sb1.tile([P, NT, NO, OW], F16, tag="out16")

    # ---- projection pieces ----
    def kv_mms(tau, fc0, fc1, pk=None):
        if pk is None:
            pk = proj_psum.tile([P, TW], F32, tag="proj")
        for fc in range(fc0, fc1):
            nc.tensor.matmul(pk[:], wkv_sb[:, fc, :],
                             xTb_sb[:, fc, TW * tau:TW * (tau + 1)],
                             start=(fc == 0), stop=(fc == FC - 1))
        return pk

    def kv_finish(tau, pk):
        c0, c1 = KPT * tau, KPT * (tau + 1)
        # k rows -> kTA directly (aligned); v rows -> vtmp for the transpose
        nc.vector.tensor_tensor(
            kTA[0:HD, c0:c1, :].rearrange("p a b -> p (a b)"),
            pk[0:HD, :], bkv_sb[0:HD, :].to_broadcast((HD, TW)),
            mybir.AluOpType.add)
        vtmp = sb2.tile([P, TW], F16, tag="vtmp")
        nc.vector.tensor_tensor(vtmp[HD:P, :], pk[HD:P, :],
                                bkv_sb[HD:P, :].to_broadcast((HD, TW)),
                                mybir.AluOpType.add)
        nc.gpsimd.dma_start(
            kTB[HD:P, c0:c1, :].rearrange("p a b -> p (a b)"),
            kTA[0:HD, c0:c1, :].rearrange("p a b -> p (a b)"))
        nc.sync.dma_start_transpose(vones[:, c0:c1, 0:HD], vtmp[HD:P, :])
        for c in range(c0, c1):
            nc.vector.tensor_tensor(
                vones[:, c, 0:66], vones[:, c, 0:66],
                maskp_sb[:, c:c + 1].to_broadcast((P, 66)),
                mybir.AluOpType.mult)

    def kv_proj(tau):
        kv_finish(tau, kv_mms(tau, 0, FC))

    def q_mms(p, fc0, fc1, pq=None):
        if pq is None:
            pq = proj_psum.tile([P, T], F32, tag="proj")
        for fc in range(fc0, fc1):
            nc.tensor.matmul(pq[:], wq_sb[:, fc, P * p:P * (p + 1)],
                             xTb_sb[:, fc, 0:T],
                             start=(fc == 0), stop=(fc == FC - 1))
        return pq

    def q_finish(p, pq):
        nc.vector.tensor_tensor(qT[:, p, :], pq[:],
                                bq_sb[:, p:p + 1].to_broadcast((P, T)),
                                mybir.AluOpType.add)

    def out_piece(cc2, g, ncc=2, epi=False):
        # one piece = ncc cc chunks accumulated in PSUM
        tt, oo = g // NO, g % NO
        if epi:  # reuse the (now idle) scores ring so pieces pipeline
            po2 = s_psum.tile([P, T], F32, tag="s", name="po2")
            po = po2[:, 0:OW]
        else:
            po = proj_psum.tile([P, OW], F32, tag="proj")
        for r in range(ncc):
            nc.tensor.matmul(po[:], ctx_all[:, cc2 + r, P * tt:P * (tt + 1)],
                             wo_sb[:, cc2 + r, OW * oo:OW * (oo + 1)],
                             start=(r == 0), stop=(r == ncc - 1))
        if cc2 == 0:
            nc.vector.tensor_tensor(out_acc[:, tt, oo, :], po[:],
                                    bob_sb[:, OW * oo:OW * (oo + 1)],
                                    mybir.AluOpType.add)
        elif cc2 + ncc < OC:
            nc.vector.tensor_tensor(out_acc[:, tt, oo, :], po[:],
                                    out_acc[:, tt, oo, :],
                                    mybir.AluOpType.add)
        else:
            nc.vector.tensor_tensor(out16[:, tt, oo, :], po[:],
                                    out_acc[:, tt, oo, :],
                                    mybir.AluOpType.add)
            eng = nc.sync if g % 2 == 0 else nc.gpsimd
            eng.dma_start(out[P * tt:P * (tt + 1), OW * oo:OW * (oo + 1)],
                          out16[:, tt, oo, :])

    # ---- deferred (wave) normalization ----
    # Per pair: cheap fp16 copy of unnormalized ctx (+ den row 64) into
    # cstage; ctx halves DMA to ctx_all. Denominators for a WAVE of pairs
    # are gathered by one strided DMA into den16 (partitions 0:16), one
    # batched DVE reciprocal serves the whole wave, and the per-query
    # reciprocal row is broadcast to 128 partitions by a tiny PE matmul
    # (stationary selector) instead of the slow gpsimd partition_broadcast.
    den16 = sb1.tile([16, T], F16, tag="den16")
    rec16 = sb1.tile([16, T], F32, tag="rec16")
    rec16h = sb1.tile([16, T], F16, tag="rec16h")
    nc.vector.memset(rec16h[:], 0.0)
    sel = sb1.tile([16, NP, P], F16, tag="sel")
    nc.gpsimd.dma_start(sel[:], selc[:])
    cstage = sb1.tile([P, 2 * NP, T], F16, tag="cstage")

    def pair_finish(p, cpA, cpB):
        # rows 0:64 ctx^T, row 64 the softmax denominator (unnormalized)
        nc.vector.tensor_scalar_mul(cstage[0:HD + 1, 2 * p, :],
                                    cpA[0:HD + 1, :], 1.0)
        nc.vector.tensor_scalar_mul(cstage[0:HD + 1, 2 * p + 1, :],
                                    cpB[0:HD + 1, :], 1.0)
        nc.gpsimd.dma_start(ctx_all[0:HD, p, :], cstage[0:HD, 2 * p, :])
        nc.gpsimd.dma_start(ctx_all[HD:P, p, :], cstage[0:HD, 2 * p + 1, :])

    def den_gather(p0, p1):
        nc.gpsimd.dma_start(den16[2 * p0:2 * p1, :],
                            cstage[HD:HD + 1, 2 * p0:2 * p1, :])

    def recip_wave(p0, p1):
        # engine APs must start at partition 0; recomputing rows [0:2*p0)
        # is free (parallel lanes)
        nc.vector.reciprocal(rec16[0:2 * p1, :], den16[0:2 * p1, :])
        nc.vector.tensor_scalar_mul(rec16h[0:2 * p1, :],
                                    rec16[0:2 * p1, :], 1.0)

    def norm_pair(q):
        rb = proj_psum.tile([P, T], F32, tag="proj")
        nc.tensor.matmul(rb[:], sel[:, q, :], rec16h[:], start=True, stop=True)
        nc.vector.tensor_tensor(ctx_all[:, q, :], ctx_all[:, q, :], rb[:],
                                mybir.AluOpType.mult)

    # ---- wave schedule: normalize pairs 0-3 during pair 4, out-proj
    # pieces stream through the j>=8 slots of pairs 5-7; pairs 4-5
    # normalize during pair 6; the rest (pairs 6-7 norm + last two
    # out-proj chunk columns) drains in the epilogue ----
    slot_tasks = {}

    def at(p, j, fn):
        slot_tasks.setdefault((p, j), []).append(fn)

    at(4, 8, lambda: den_gather(0, 4))
    at(4, 9, lambda: recip_wave(0, 4))
    for q in range(4):
        at(4, 10 + q, lambda q=q: norm_pair(q))
    for g in range(8):
        at(5, 8 + g, lambda g=g: out_piece(0, g))
    at(6, 8, lambda: den_gather(4, 6))
    at(6, 9, lambda: recip_wave(4, 6))
    at(6, 10, lambda: norm_pair(4))
    at(6, 11, lambda: norm_pair(5))
    for g in range(8):
        p_, j_ = (6, 12 + g) if g < 4 else (7, 8 + g - 4)
        at(p_, j_, lambda g=g: out_piece(2, g))
    for g in range(4):
        at(7, 12 + g, lambda g=g: out_piece(4, g))

    # ---- prologue: window 0 -> kv(0) and q(0); remaining windows follow
    # the first v-transpose on the SP queue ----
    kv_proj(0)
    nc.gpsimd.dma_start(wq_sb[:, :, P:4 * P], wq_r[:, :, P:4 * P])
    q_finish(0, q_mms(0, 0, FC))
    for w in range(1, NW):
        x_window(w)
    nc.sync.dma_start(bob_sb[:], bo_r.to_broadcast((P, H)))

    # ---- attention: pairs x key chunks, ctx lags scores by two chunks ----
    for p in range(NP):
        cpA = c_psum.tile([P, T], F32, tag="ctx")
        cpB = c_psum.tile([P, T], F32, tag="ctx")
        lagq = []
        kv_pk = None
        q_pq = None
        SW = 64 if "smallscores" in ABLATE else T
        EW = 64 if "smallexp" in ABLATE else 2 * T
        for j in range(KC):
            # both heads' scores in one 2-bank psum tile -> single exp pass
            sp = s_psum.tile([P, 2 * T], F32, tag="s")
            nc.tensor.matmul(sp[:, 0:SW], kTA[:, j, :], qT[:, p, 0:SW],
                             start=True, stop=True)
            nc.tensor.matmul(sp[:, T:T + SW], kTB[:, j, :], qT[:, p, 0:SW],
                             start=True, stop=True)
            pT = sb3.tile([P, 2 * T], F16, tag="pT")
            nc.scalar.activation(pT[:, 0:EW], sp[:, 0:EW],
                                 mybir.ActivationFunctionType.Exp,
                                 scale=scale)
            pTA = pT[:, 0:T]
            pTB = pT[:, T:2 * T]
            # interleaved projection work (keeps PE busy during the exp);
            # the single proj bank is time-shared: kv/q in the first half of
            # the chunk loop, out-proj pieces in the second half
            if p == 0:
                if j in (0, 1, 2, 3, 6, 7):
                    tau = {0: 1, 1: 1, 2: 2, 3: 2, 6: 3, 7: 3}[j]
                    if j % 2 == 0:
                        kv_pk = kv_mms(tau, 0, FC // 2)
                    else:
                        kv_mms(tau, FC // 2, FC, kv_pk)
                        kv_finish(tau, kv_pk)
                if j == 8:
                    nc.sync.dma_start(
                        wo_sb[:], wo.rearrange("(fo p) o -> p fo o", p=P))
                if j == 12:
                    nc.sync.dma_start(wq_sb[:, :, 4 * P:H],
                                      wq_r[:, :, 4 * P:H])
                if j >= 8:
                    q_pq = q_mms(1, j - 8, j - 7, q_pq if j > 8 else None)
                    if j == 15:
                        q_finish(1, q_pq)
            elif "noproj" not in ABLATE:
                if j < 8 and p < NP - 1:
                    q_pq = q_mms(p + 1, j, j + 1, q_pq if j > 0 else None)
                    if j == 7:
                        q_finish(p + 1, q_pq)
                if j >= 8:
                    for task in slot_tasks.get((p, j), ()):
                        task()
            lagq.append((pTA, pTB, j))
            CW = 64 if "smallctx" in ABLATE else T
            del pT, pTA, pTB
            if len(lagq) > 2:
                pA, pB, jp = lagq.pop(0)
                nc.tensor.matmul(cpA[0:HD + 1, 0:CW], vones[:, jp, 0:HD + 1],
                                 pA[:, 0:CW], start=(jp == 0), stop=False)
                nc.tensor.matmul(cpB[0:HD + 1, 0:CW], vones[:, jp, 0:HD + 1],
                                 pB[:, 0:CW], start=(jp == 0), stop=False)
        while lagq:
            pA, pB, jp = lagq.pop(0)
            nc.tensor.matmul(cpA[0:HD + 1, 0:CW], vones[:, jp, 0:HD + 1],
                             pA[:, 0:CW], start=(jp == 0), stop=(jp == KC - 1))
            nc.tensor.matmul(cpB[0:HD + 1, 0:CW], vones[:, jp, 0:HD + 1],
                             pB[:, 0:CW], start=(jp == 0), stop=(jp == KC - 1))
        pair_finish(p, cpA, cpB)

    # ---- epilogue: finish cc2=4 pieces, normalize pairs 6-7, then the
    # final out-projection chunk pair streams straight to DRAM
    # (pipelined through the now-idle scores banks) ----
    for g in range(4, 8):
        out_piece(4, g, epi=True)
    den_gather(6, 8)
    recip_wave(6, 8)
    norm_pair(6)
    norm_pair(7)
    for g in range(NT * NO):
        out_piece(OC - 2, g, epi=True)

    if DEBUG_DUMP:
        for name, t in [("dbg_kTA", kTA), ("dbg_kTB", kTB),
                        ("dbg_vones", vones), ("dbg_qT", qT),
                        ("dbg_ctx", ctx_all)]:
            shp = [int(s) for s in t.shape]
            d = nc.dram_tensor(name, shp, F16, kind="ExternalOutput").ap()
            nc.sync.dma_start(d[:], t[:])
        dacc = nc.dram_tensor("dbg_acc", [P, NT, NO, OW], F32,
                              kind="ExternalOutput").ap()
        nc.sync.dma_start(dacc[:], out_acc[:])


# ---------------- host side ----------------

_RUNNER_CACHE = {}


def _get_runner(reps=1):
    key = reps
    if key not in _RUNNER_CACHE:
        from runner import make_runner  # dev only; grading uses the fallback
        nc = build_nc(reps=reps)
        _RUNNER_CACHE[key] = (nc, make_runner(nc, NCORES))
    return _RUNNER_CACHE[key]


def _prep_in_maps(hidden_state, attention_mask, Wq, bq, Wk, bk, Wv, bv, Wo, bo):
    f16 = np.float16
    hid = np.asarray(hidden_state, np.float32)
    mask = np.asarray(attention_mask, np.float32)
    hT = np.ascontiguousarray(hid.transpose(0, 2, 1)).astype(f16)  # [B, H, S]
    wkv = np.concatenate([np.asarray(Wk, np.float32),
                          np.asarray(Wv, np.float32)], axis=1).astype(f16)
    wq_b = np.asarray(Wq, np.float32).astype(f16)
    wo_b = np.asarray(Wo, np.float32).astype(f16)
    bq_p = np.asarray(bq, np.float32).reshape(H // 128, 128).T.copy()
    bkv_p = np.concatenate([np.asarray(bk, np.float32),
                            np.asarray(bv, np.float32)]).reshape(128, 1)
    bo_r = np.asarray(bo, np.float32).reshape(1, H).copy()
    selc = np.zeros((16, NH // 2, 128), np.float16)
    for q in range(NH // 2):
        selc[2 * q, q, 0:HD] = 1.0
        selc[2 * q + 1, q, HD:128] = 1.0
    in_maps = []
    for c in range(NCORES):
        b = c // CORES_PER_BATCH
        s0 = (c % CORES_PER_BATCH) * T
        xrot = np.ascontiguousarray(np.roll(hT[b], -s0, axis=1))
        mrot = np.roll(mask[b], -s0).reshape(S // 128, 128).T.copy()
        in_maps.append({
            "xT": xrot, "wkv": wkv, "wq": wq_b, "wo": wo_b,
            "bq_p": bq_p, "bkv_p": bkv_p, "bo_r": bo_r,
            "maskp": mrot.astype(np.float16), "selc": selc,
        })
    return in_maps


def kernel(hidden_state, attention_mask, Wq, bq, Wk, bk, Wv, bv, Wo, bo):
    in_maps = _prep_in_maps(hidden_state, attention_mask,
                            Wq, bq, Wk, bk, Wv, bv, Wo, bo)
    try:
        nc, runner = _get_runner()
        args = runner.put(runner.pack(in_maps))
        outs = runner(args)
        res = runner.unpack(outs)
    except ImportError:
        from concourse.bass_utils import run_bass_kernel_spmd
        nc = build_nc()
        res = run_bass_kernel_spmd(nc, in_maps, list(range(NCORES))).results
    full = np.empty((B, S, H), np.float32)
    for c in range(NCORES):
        b = c // CORES_PER_BATCH
        s0 = (c % CORES_PER_BATCH) * T
        full[b, s0:s0 + T] = res[c]["out"].astype(np.float32)
    return full

